# revision 1
# baseline (speedup 1.0000x reference)
"""Trainium2 Bass kernel for nn_CustomCNN (LeNet-style CNN, batch 8192).

Strategy (pure data parallel over 8 cores, 1024 images each, 8 blocks of 128):
- x loaded batch-major [128 imgs (partitions), 3072 feats] -> perfect HBM bursts.
- PE transposes 128x128 tiles to feature-major chunks (f32r, 1.5 cyc/row).
- conv1 via batch-in-M matmuls: out[128 imgs, 336 outs] = chunk[128 feat, 128 img].T
  @ W[128 feat, 336 outs]; W are prebuilt sparse conv matrices; 6 PSUM-accumulated
  matmuls (3 channels x 2 row-chunk halves) per output tile. f32r moving, N>=256
  -> 1 cycle/row on the PE.
- tanh on ACT straight out of PSUM.
- The bugged avgpool (channel-mean + 2x2) is pure adds on DVE via rearranged APs;
  scale factors folded into the next layer's weights.
- conv2/conv3 collapse: pooled output is channel-replicated, so k2/k3 collapse to
  single-channel kernels (sum over in-channels). conv3 degenerates to a 25->120
  matmul; then FC layers with bias via augmented ones-row.
"""

import os
import sys
import numpy as np

if "/opt/trn_rl_repo" not in sys.path:
    sys.path.insert(0, "/opt/trn_rl_repo")

NCORES = 8
BPC = 1024          # images per core
NBLK = 8            # blocks of 128 images per core
P = 128

_CACHE = {}


def _build_weight_mats(k1, k2, k3, W1, b1, W2, b2):
    """Host-side construction of the dense matmul operand matrices."""
    f32 = np.float32
    k1 = np.asarray(k1, f32)
    k2e = (np.asarray(k2, f32).sum(1) / 24.0).astype(f32)   # [16,5,5] pool1 scale folded
    k3e = (np.asarray(k3, f32).sum(1) / 64.0).astype(f32)   # [120,5,5] pool2 scale folded

    # conv1: W1m[c, d, half, row=(rt*32+w), col=(ocl*112 + ohl*28 + ow)]
    # chunk rows are 4-row groups of one channel; d=0 -> rows 4a..4a+3, d=1 -> 4a+4..4a+7
    W1m = np.zeros((3, 2, 2, 128, 336), f32)
    for c in range(3):
        for d in range(2):
            for half in range(2):
                for ocl in range(3):
                    oc = half * 3 + ocl
                    for ohl in range(4):
                        for ow in range(28):
                            col = ocl * 112 + ohl * 28 + ow
                            for rt in range(4):
                                kh = rt + 4 * d - ohl
                                if 0 <= kh < 5:
                                    for kw in range(5):
                                        W1m[c, d, half, rt * 32 + ow + kw, col] = k1[oc, c, kh, kw]

    # conv2 (collapsed): input s1 [14,14]; chunk = 9 rows x 14 cols = 126 feats.
    # W2m[row=(rt*14+w), col=(oc*50 + ohl*10 + ow)] ; rows 126/127 zero-padded.
    W2m = np.zeros((128, 800), f32)
    for oc in range(16):
        for ohl in range(5):
            for ow in range(10):
                col = oc * 50 + ohl * 10 + ow
                for kh in range(5):
                    rt = ohl + kh          # 0..8
                    for kw in range(5):
                        W2m[rt * 14 + ow + kw, col] = k2e[oc, kh, kw]

    # conv3 (collapsed to matmul): s2 [25] -> 120
    K3m = np.zeros((25, 120), f32)
    for o in range(120):
        K3m[:, o] = k3e[o].reshape(25)

    W1a = np.zeros((121, 84), f32)
    W1a[:120] = np.asarray(W1, f32)
    W1a[120] = np.asarray(b1, f32)
    W2a = np.zeros((85, 10), f32)
    W2a[:84] = np.asarray(W2, f32)
    W2a[84] = np.asarray(b2, f32)

    return {
        "w1m": W1m,
        "w2m": W2m,
        "k3m": K3m,
        "fc1": W1a,
        "fc2": W2a,
        "ident": np.eye(128, dtype=f32),
        "ones": np.ones((128, 1), dtype=f32),
    }


def _build_bass(n_blocks=NBLK, n_reps=1):
    import concourse.bass as bass
    import concourse.bacc as bacc
    import concourse.mybir as mybir
    import concourse.tile as tile

    f32 = mybir.dt.float32
    f32r = mybir.dt.float32r
    TANH = mybir.ActivationFunctionType.Tanh
    MS = bass.MemorySpace

    nc = bacc.Bacc("TRN2", target_bir_lowering=False, debug=False,
                   num_devices=NCORES)

    bpc = n_blocks * P
    x_d = nc.dram_tensor("x", [3072, bpc], f32r, kind="ExternalInput")
    w1_d = nc.dram_tensor("w1m", [3, 2, 2, 128, 336], f32r, kind="ExternalInput")
    w2_d = nc.dram_tensor("w2m", [128, 800], f32r, kind="ExternalInput")
    k3_d = nc.dram_tensor("k3m", [25, 120], f32r, kind="ExternalInput")
    fc1_d = nc.dram_tensor("fc1", [121, 84], f32r, kind="ExternalInput")
    fc2_d = nc.dram_tensor("fc2", [85, 10], f32r, kind="ExternalInput")
    id_d = nc.dram_tensor("ident", [128, 128], f32r, kind="ExternalInput")
    ones_d = nc.dram_tensor("ones", [128, 1], f32r, kind="ExternalInput")
    out_d = nc.dram_tensor("out", [bpc, 10], f32, kind="ExternalOutput")

    with tile.TileContext(nc) as tc:
        with (
            tc.tile_pool(name="consts", bufs=1) as consts,
            tc.tile_pool(name="chk", bufs=2) as chk,
            tc.tile_pool(name="act", bufs=2) as actp,
            tc.tile_pool(name="tmp", bufs=2) as tmp,
            tc.tile_pool(name="outp", bufs=1) as outp,
            tc.tile_pool(name="pst", bufs=1, space=MS.PSUM) as pst,
            tc.tile_pool(name="ps1", bufs=2, space=MS.PSUM) as ps1p,
            tc.tile_pool(name="ps2", bufs=1, space=MS.PSUM) as ps2p,
            tc.tile_pool(name="ps3", bufs=1, space=MS.PSUM) as ps3p,
        ):
            # ---- constants into SBUF (once) ----
            w1sb = consts.tile([128, 12 * 336], f32r, tag="w1sb")
            for c in range(3):
                for d in range(2):
                    for h in range(2):
                        k = (c * 2 + d) * 2 + h
                        nc.sync.dma_start(w1sb[:, k * 336:(k + 1) * 336], w1_d[c, d, h])
            w2sb = consts.tile([128, 800], f32r, tag="w2sb")
            nc.sync.dma_start(w2sb[:], w2_d[:])
            k3sb = consts.tile([128, 120], f32r, tag="k3sb")
            nc.sync.dma_start(k3sb[0:25, :], k3_d[:])
            fc1sb = consts.tile([128, 84], f32r, tag="fc1sb")
            nc.sync.dma_start(fc1sb[0:121, :], fc1_d[:])
            fc2sb = consts.tile([128, 10], f32r, tag="fc2sb")
            nc.sync.dma_start(fc2sb[0:85, :], fc2_d[:])
            ident = consts.tile([128, 128], f32r, tag="ident")
            nc.sync.dma_start(ident[:], id_d[:])
            onescol = consts.tile([128, 1], f32r, tag="onescol")
            nc.sync.dma_start(onescol[:], ones_d[:])
            out_sb = outp.tile([128, n_blocks * 10], f32, tag="outsb")

            def w1t(c, d, h):
                k = (c * 2 + d) * 2 + h
                return w1sb[:, k * 336:(k + 1) * 336]

            xr = x_d[:].rearrange("(k p) n -> p k n", p=128)
            for blk in range(n_blocks * n_reps):
                blk = blk % n_blocks
                # ---- load 128 images, already feature-major from host ----
                chunks = chk.tile([128, 3072], f32r, tag="chunks")
                c3 = chunks[:].rearrange("p (k i) -> p k i", i=128)
                nc.sync.dma_start(c3, xr[:, :, blk * P:(blk + 1) * P])

                def chunk(c, g):   # channel c, 4-row group g (0..7)
                    k = c * 8 + g
                    return chunks[:, k * 128:(k + 1) * 128]

                # ---- conv1 + tanh1 -> t1 [128, 6*28*28] (oc, oh, ow) ----
                t1 = actp.tile([128, 4704], f32, tag="t1")
                t1r = t1[:].rearrange("p (oc oh ow) -> p oc oh ow", oh=28, ow=28)
                t1h = t1[:].rearrange("p (h ocl oh ow) -> p h ocl oh ow",
                                      ocl=3, oh=28, ow=28)
                for a in range(7):
                    ps = ps1p.tile([128, 1024], f32, tag="ps1")  # 2 banks
                    for h in range(2):
                        for d in range(2):
                            for c in range(3):
                                nc.tensor.matmul(
                                    ps[:, h * 512:h * 512 + 336],
                                    chunk(c, a + d), w1t(c, d, h),
                                    start=(d == 0 and c == 0),
                                    stop=(d == 1 and c == 2),
                                )
                    dst = t1h[:, :, :, 4 * a:4 * a + 4, :]
                    srcap = ps[:].rearrange("p (h x) -> p h x", h=2)[:, :, 0:336]
                    srcap = srcap.rearrange("p h (ocl oh ow) -> p h ocl oh ow", oh=4, ow=28)
                    nc.scalar.activation(dst, srcap, TANH)

                # ---- pool1: sum 6 channels + 2x2 sum (scale folded into W2m) ----
                u = tmp.tile([128, 784], f32, tag="u")
                nc.vector.tensor_add(u[:], t1[:, 0:784], t1[:, 784:1568])
                for c in range(2, 6):
                    nc.vector.tensor_add(u[:], u[:], t1[:, c * 784:(c + 1) * 784])
                ur = u[:].rearrange("p (i t w) -> p t i w", t=2, w=28)  # i=14
                v = tmp.tile([128, 392], f32, tag="v")                  # [14, 28]
                vr = v[:].rearrange("p (i w) -> p i w", w=28)
                nc.vector.tensor_add(vr, ur[:, 0], ur[:, 1])
                v2 = v[:].rearrange("p (i j t) -> p t i j", t=2, j=14)
                s1 = tmp.tile([128, 196], f32r, tag="s1")               # [14, 14]
                s1r = s1[:].rearrange("p (i j) -> p i j", j=14)
                nc.vector.tensor_add(s1r, v2[:, 0], v2[:, 1])

                # ---- conv2 + tanh2 -> t2 [128, 16*10*10] ----
                t2 = actp.tile([128, 1600], f32, tag="t2")
                t2r = t2[:].rearrange("p (oc oh ow) -> p oc oh ow", oh=10, ow=10)
                for ch in range(2):
                    ptc = pst.tile([128, 128], f32r, tag="pt")
                    # s1 rows ch*5 .. ch*5+8 -> cols ch*70 .. ch*70+126
                    nc.tensor.transpose(ptc[0:126, :], s1[:, ch * 70:ch * 70 + 126], ident[:])
                    s1T = tmp.tile([128, 128], f32r, tag="s1T")
                    nc.vector.tensor_copy(s1T[0:126, :], ptc[0:126, :])
                    ps2 = ps2p.tile([128, 1024], f32, tag="ps2")  # 2 banks
                    for h2 in range(2):
                        nc.tensor.matmul(ps2[:, h2 * 512:h2 * 512 + 400], s1T[0:126, :],
                                         w2sb[0:126, h2 * 400:(h2 + 1) * 400])
                    t2h = t2[:].rearrange("p (h ocl oh ow) -> p h ocl oh ow",
                                          ocl=8, oh=10, ow=10)
                    dst = t2h[:, :, :, 5 * ch:5 * ch + 5, :]
                    srcap = ps2[:].rearrange("p (h x) -> p h x", h=2)[:, :, 0:400]
                    srcap = srcap.rearrange("p h (ocl oh ow) -> p h ocl oh ow", oh=5, ow=10)
                    nc.scalar.activation(dst, srcap, TANH)

                # ---- pool2: sum 16 channels + 2x2 (scale folded into K3m) ----
                u2 = tmp.tile([128, 100], f32, tag="u2")
                nc.vector.tensor_add(u2[:], t2[:, 0:100], t2[:, 100:200])
                for c in range(2, 16):
                    nc.vector.tensor_add(u2[:], u2[:], t2[:, c * 100:(c + 1) * 100])
                u2r = u2[:].rearrange("p (i t w) -> p t i w", t=2, w=10)  # i=5
                v2t = tmp.tile([128, 50], f32, tag="v2t")                 # [5, 10]
                v2r = v2t[:].rearrange("p (i w) -> p i w", w=10)
                nc.vector.tensor_add(v2r, u2r[:, 0], u2r[:, 1])
                v3 = v2t[:].rearrange("p (i j t) -> p t i j", t=2, j=5)
                s2 = tmp.tile([128, 32], f32r, tag="s2")                  # [5,5] in 0:25
                s2r = s2[:, 0:25].rearrange("p (i j) -> p i j", j=5)
                nc.vector.tensor_add(s2r, v3[:, 0], v3[:, 1])

                # ---- conv3 (25->120) + tanh3 ----
                pt3 = pst.tile([128, 128], f32r, tag="pt")
                nc.tensor.transpose(pt3[0:25, :], s2[:, 0:25], ident[:])
                s2T = tmp.tile([128, 128], f32r, tag="s2T")
                nc.vector.tensor_copy(s2T[0:25, :], pt3[0:25, :])
                ps3 = ps3p.tile([128, 120], f32, tag="pstail")
                nc.tensor.matmul(ps3[:], s2T[0:25, :], k3sb[0:25, :])
                t3 = tmp.tile([128, 128], f32r, tag="t3")
                nc.scalar.activation(t3[:, 0:120], ps3[:], TANH)
                nc.vector.tensor_copy(t3[:, 120:121], onescol[:])  # ones col -> ones row after T

                # ---- fc1 + tanh4 ----
                pt4 = pst.tile([128, 128], f32r, tag="pt")
                nc.tensor.transpose(pt4[0:121, :], t3[:, 0:121], ident[:])
                t3a = tmp.tile([128, 128], f32r, tag="t3a")
                nc.vector.tensor_copy(t3a[0:121, :], pt4[0:121, :])
                ps4t = ps3p.tile([128, 120], f32, tag="pstail")
                ps4 = ps4t[:, 0:84]
                nc.tensor.matmul(ps4[:], t3a[0:121, :], fc1sb[0:121, :])
                t4 = tmp.tile([128, 128], f32r, tag="t4")
                nc.scalar.activation(t4[:, 0:84], ps4[:], TANH)
                nc.vector.tensor_copy(t4[:, 84:85], onescol[:])

                # ---- fc2 ----
                pt5 = pst.tile([128, 128], f32r, tag="pt")
                nc.tensor.transpose(pt5[0:85, :], t4[:, 0:85], ident[:])
                t4a = tmp.tile([128, 128], f32r, tag="t4a")
                nc.vector.tensor_copy(t4a[0:85, :], pt5[0:85, :])
                ps5t = ps3p.tile([128, 120], f32, tag="pstail")
                ps5 = ps5t[:, 0:10]
                nc.tensor.matmul(ps5[:], t4a[0:85, :], fc2sb[0:85, :])
                nc.vector.tensor_copy(out_sb[:, blk * 10:(blk + 1) * 10], ps5[:])

            # ---- one output DMA: SBUF [128, nblk*10] -> DRAM [nblk*128, 10] ----
            od = out_d[:].rearrange("(blk p) f -> p blk f", p=P)
            ob = out_sb[:].rearrange("p (blk f) -> p blk f", f=10)
            nc.sync.dma_start(od, ob)

    nc.compile()
    return nc


def _get_nc(n_blocks=NBLK, n_reps=1):
    key = ("nc", n_blocks, n_reps)
    if key not in _CACHE:
        _CACHE[key] = _build_bass(n_blocks, n_reps)
    return _CACHE[key]


def kernel(n_reps=1, **inputs):
    x = np.asarray(inputs["x"], np.float32)
    wm = _build_weight_mats(inputs["k1"], inputs["k2"], inputs["k3"],
                            inputs["W1"], inputs["b1"], inputs["W2"], inputs["b2"])
    nc = _get_nc(NBLK, n_reps)

    from concourse.bass_utils import run_bass_kernel_spmd

    in_maps = []
    for core in range(NCORES):
        xc = np.ascontiguousarray(
            x[core * BPC:(core + 1) * BPC].reshape(BPC, 3072).T)
        m = {"x": xc}
        m.update(wm)
        in_maps.append(m)

    res = run_bass_kernel_spmd(nc, in_maps, core_ids=list(range(NCORES)))
    _CACHE["last_result"] = res
    out = np.concatenate([r["out"] for r in res.results], axis=0)
    return out.astype(np.float32)



# revision 4
# speedup vs baseline: 2.5639x; 2.5639x over previous
"""Trainium2 Bass kernel for nn_CustomCNN (LeNet-style CNN, batch 8192).

Strategy (pure data parallel over 8 cores, 1024 images each, 8 blocks of 128):
- x prepped host-side to fp16, feature-major, one contiguous 6KB line per
  partition per block -> peak DMA efficiency.
- conv1 via batch-in-M matmuls (images stationary, sparse conv weight matrix
  moving), all bf16 (1 cyc/col any N; LDWEIGHTS hoistable ahead of matmuls).
- tanh on ACT straight out of PSUM, bf16 outputs.
- Bugged avgpool (channel-mean + 2x2) as bf16 DVE add-trees (2x packed mode);
  scale factors folded into next layer's weights.
- conv2/conv3 collapse to single-channel kernels (pooled output is
  channel-replicated). conv3+fc1 batched across all 8 blocks feature-major
  (no per-block transposes for the FC stack); fc2 per block image-major so
  the final layout needs no transpose.
- Software pipeline: block b's mid-stage (transpose/conv2/pool2) is emitted
  after block b+1's conv1, so no engine queue head-of-line blocking.
"""

import sys

if "/opt/trn_rl_repo" not in sys.path:
    sys.path.insert(0, "/opt/trn_rl_repo")

import numpy as np
BF16 = np.float16

NCORES = 8
BPC = 1024          # images per core
NBLK = 8            # blocks of 128 images per core
P = 128

_CACHE = {}


def _prep_x(x, core):
    """[8192,3,32,32] fp32 -> per-core [128, NBLK*3072] bf16, layout
    [p, blk, k, i] with feature f = k*128+p, image = blk*128+i."""
    xc = np.asarray(x, np.float32)[core * BPC:(core + 1) * BPC]
    xc = xc.reshape(NBLK, P, 24, P)            # [blk, i, k, p]
    xc = np.ascontiguousarray(xc.transpose(3, 0, 2, 1))  # [p, blk, k, i]
    return xc.reshape(P, NBLK * 3072).astype(BF16)


def _build_weight_mats(k1, k2, k3, W1, b1, W2, b2):
    """Host-side construction of the dense matmul operand matrices."""
    f32 = np.float32
    k1 = np.asarray(k1, f32)
    k2e = (np.asarray(k2, f32).sum(1) / 24.0).astype(f32)   # [16,5,5] pool1 scale folded
    k3e = (np.asarray(k3, f32).sum(1) / 64.0).astype(f32)   # [120,5,5] pool2 scale folded

    # conv1: W1m[c, d, half, row=(rt*32+w), col=(ocl*112 + ohl*28 + ow)]
    # chunk rows are 4-row groups of one channel; d=0 -> rows 4a..4a+3, d=1 -> 4a+4..4a+7
    W1m = np.zeros((3, 2, 2, 128, 336), f32)
    for c in range(3):
        for d in range(2):
            for half in range(2):
                for ocl in range(3):
                    oc = half * 3 + ocl
                    for ohl in range(4):
                        for ow in range(28):
                            col = ocl * 112 + ohl * 28 + ow
                            for rt in range(4):
                                kh = rt + 4 * d - ohl
                                if 0 <= kh < 5:
                                    for kw in range(5):
                                        W1m[c, d, half, rt * 32 + ow + kw, col] = k1[oc, c, kh, kw]

    # conv2 (collapsed): input s1 [14,14]; chunk = 9 rows x 14 cols = 126 feats.
    # W2m[row=(rt*14+w), col=(oc*50 + ohl*10 + ow)] ; rows 126/127 zero-padded.
    W2m = np.zeros((128, 800), f32)
    for oc in range(16):
        for ohl in range(5):
            for ow in range(10):
                col = oc * 50 + ohl * 10 + ow
                for kh in range(5):
                    rt = ohl + kh          # 0..8
                    for kw in range(5):
                        W2m[rt * 14 + ow + kw, col] = k2e[oc, kh, kw]

    # conv3 (collapsed to matmul): s2 [25] -> 120
    K3m = np.zeros((25, 120), f32)
    for o in range(120):
        K3m[:, o] = k3e[o].reshape(25)

    W1a = np.zeros((121, 84), f32)
    W1a[:120] = np.asarray(W1, f32)
    W1a[120] = np.asarray(b1, f32)
    W2a = np.zeros((85, 10), f32)
    W2a[:84] = np.asarray(W2, f32)
    W2a[84] = np.asarray(b2, f32)

    return {
        "w1m": W1m.astype(BF16),
        "w2m": W2m.astype(BF16),
        "k3m": K3m.astype(BF16),
        "fc1": W1a.astype(BF16),
        "fc2": W2a.astype(BF16),
        "ident": np.eye(128, dtype=f32).astype(BF16),
        "ones": np.ones((1, NBLK * P), f32).astype(BF16),
    }


def _build_bass(n_blocks=NBLK, n_reps=1):
    import concourse.bass as bass
    import concourse.bacc as bacc
    import concourse.mybir as mybir
    import concourse.tile as tile

    f32 = mybir.dt.float32
    bf16 = mybir.dt.float16
    TANH = mybir.ActivationFunctionType.Tanh
    MS = bass.MemorySpace

    nc = bacc.Bacc("TRN2", target_bir_lowering=False, debug=False,
                   num_devices=NCORES)

    bpc = n_blocks * P
    x_d = nc.dram_tensor("x", [P, n_blocks * 3072], bf16, kind="ExternalInput")
    w1_d = nc.dram_tensor("w1m", [3, 2, 2, 128, 336], bf16, kind="ExternalInput")
    w2_d = nc.dram_tensor("w2m", [128, 800], bf16, kind="ExternalInput")
    k3_d = nc.dram_tensor("k3m", [25, 120], bf16, kind="ExternalInput")
    fc1_d = nc.dram_tensor("fc1", [121, 84], bf16, kind="ExternalInput")
    fc2_d = nc.dram_tensor("fc2", [85, 10], bf16, kind="ExternalInput")
    id_d = nc.dram_tensor("ident", [128, 128], bf16, kind="ExternalInput")
    ones_d = nc.dram_tensor("ones", [1, NBLK * P], bf16, kind="ExternalInput")
    out_d = nc.dram_tensor("out", [bpc, 10], f32, kind="ExternalOutput")

    with tile.TileContext(nc) as tc:
        with (
            tc.tile_pool(name="consts", bufs=1) as consts,
            tc.tile_pool(name="chk", bufs=3) as chk,
            tc.tile_pool(name="act", bufs=2) as actp,
            tc.tile_pool(name="tmp", bufs=2) as tmp,
            tc.tile_pool(name="s1p", bufs=2) as s1p,
            tc.tile_pool(name="outp", bufs=1) as outp,
            tc.tile_pool(name="ps1", bufs=2, space=MS.PSUM) as ps1p,
            tc.tile_pool(name="pst", bufs=2, space=MS.PSUM) as pstp,
            tc.tile_pool(name="ps2", bufs=1, space=MS.PSUM) as ps2p,
        ):
            # ---- constants into SBUF (once) ----
            w1sb = consts.tile([128, 12 * 336], bf16, tag="w1sb")
            for c in range(3):
                for d in range(2):
                    for h in range(2):
                        k = (c * 2 + d) * 2 + h
                        nc.sync.dma_start(w1sb[:, k * 336:(k + 1) * 336], w1_d[c, d, h])
            w2sb = consts.tile([128, 800], bf16, tag="w2sb")
            nc.sync.dma_start(w2sb[:], w2_d[:])
            k3sb = consts.tile([128, 120], bf16, tag="k3sb")
            nc.sync.dma_start(k3sb[0:25, :], k3_d[:])
            fc1sb = consts.tile([128, 84], bf16, tag="fc1sb")
            nc.sync.dma_start(fc1sb[0:121, :], fc1_d[:])
            fc2sb = consts.tile([128, 10], bf16, tag="fc2sb")
            nc.sync.dma_start(fc2sb[0:85, :], fc2_d[:])
            ident = consts.tile([128, 128], bf16, tag="ident")
            nc.sync.dma_start(ident[:], id_d[:])
            out_sb = outp.tile([128, n_blocks * 10], f32, tag="outsb")
            # feature-major FC stack tiles (persist across blocks)
            s2T_all = outp.tile([128, bpc], bf16, tag="s2T_all")   # rows 0:25
            t3f = outp.tile([128, bpc], bf16, tag="t3f")           # rows 0:121
            t4f = outp.tile([128, bpc], bf16, tag="t4f")           # rows 0:85
            nc.sync.dma_start(t3f[120:121, :], ones_d[:, 0:bpc])
            nc.sync.dma_start(t4f[84:85, :], ones_d[:, 0:bpc])

            def w1t(c, d, h):
                k = (c * 2 + d) * 2 + h
                return w1sb[:, k * 336:(k + 1) * 336]

            def emit_front(blk):
                """DMA + conv1 + tanh1 + pool1 -> s1 tile (bf16). Returns s1."""
                # ---- load 128 images, feature-major contiguous ----
                chunks = chk.tile([128, 3072], bf16, tag="chunks")
                nc.sync.dma_start(chunks[:], x_d[:, blk * 3072:(blk + 1) * 3072])

                def chunk(c, g):   # channel c, 4-row group g (0..7)
                    k = c * 8 + g
                    return chunks[:, k * 128:(k + 1) * 128]

                # ---- conv1 + tanh1 -> t1 [128, 6*28*28] (oc, oh, ow) ----
                t1 = actp.tile([128, 4704], bf16, tag="t1")
                t1h = t1[:].rearrange("p (h ocl oh ow) -> p h ocl oh ow",
                                      ocl=3, oh=28, ow=28)
                for a in range(7):
                    ps = ps1p.tile([128, 1024], f32, tag="ps1")  # 2 banks
                    for d in range(2):
                        for c in range(3):
                            for h in range(2):
                                nc.tensor.matmul(
                                    ps[:, h * 512:h * 512 + 336],
                                    chunk(c, a + d), w1t(c, d, h),
                                    start=(d == 0 and c == 0),
                                    stop=(d == 1 and c == 2),
                                )
                    dst = t1h[:, :, :, 4 * a:4 * a + 4, :]
                    srcap = ps[:].rearrange("p (h x) -> p h x", h=2)[:, :, 0:336]
                    srcap = srcap.rearrange("p h (ocl oh ow) -> p h ocl oh ow", oh=4, ow=28)
                    nc.scalar.activation(dst, srcap, TANH)

                # ---- pool1: sum 6 channels (tree) + 2x2 sum; scales folded in W2m ----
                u = tmp.tile([128, 2352], bf16, tag="u")
                nc.vector.tensor_add(u[:], t1[:, 0:2352], t1[:, 2352:4704])
                u2 = tmp.tile([128, 784], bf16, tag="uu")
                nc.vector.tensor_add(u2[:], u[:, 0:784], u[:, 784:1568])
                nc.vector.tensor_add(u2[:], u2[:], u[:, 1568:2352])
                ur = u2[:].rearrange("p (i t w) -> p t i w", t=2, w=28)  # i=14
                v = tmp.tile([128, 392], bf16, tag="v")                  # [14, 28]
                vr = v[:].rearrange("p (i w) -> p i w", w=28)
                nc.vector.tensor_add(vr, ur[:, 0], ur[:, 1])
                v2 = v[:].rearrange("p (i j t) -> p t i j", t=2, j=14)
                s1 = s1p.tile([128, 196], bf16, tag="s1")               # [14, 14]
                s1r = s1[:].rearrange("p (i j) -> p i j", j=14)
                nc.vector.tensor_add(s1r, v2[:, 0], v2[:, 1])
                return s1

            def emit_mid(blk, s1):
                """transpose + conv2 + tanh2 + pool2 + transpose -> s2T_all cols."""
                # ---- conv2 + tanh2 -> t2 [128, 16*10*10] ----
                t2 = actp.tile([128, 1600], bf16, tag="t2")
                for ch in range(2):
                    ptc = pstp.tile([128, 128], bf16, tag="ptc")
                    nc.tensor.transpose(ptc[0:126, :], s1[:, ch * 70:ch * 70 + 126], ident[:])
                    s1T = tmp.tile([128, 128], bf16, tag="s1T")
                    nc.vector.tensor_copy(s1T[0:126, :], ptc[0:126, :])
                    ps2 = ps2p.tile([128, 1024], f32, tag="ps2")  # 2 banks
                    for h2 in range(2):
                        nc.tensor.matmul(ps2[:, h2 * 512:h2 * 512 + 400], s1T[0:126, :],
                                         w2sb[0:126, h2 * 400:(h2 + 1) * 400])
                    t2h = t2[:].rearrange("p (h ocl oh ow) -> p h ocl oh ow",
                                          ocl=8, oh=10, ow=10)
                    dst = t2h[:, :, :, 5 * ch:5 * ch + 5, :]
                    srcap = ps2[:].rearrange("p (h x) -> p h x", h=2)[:, :, 0:400]
                    srcap = srcap.rearrange("p h (ocl oh ow) -> p h ocl oh ow", oh=5, ow=10)
                    nc.scalar.activation(dst, srcap, TANH)

                # ---- pool2: sum 16 channels (tree) + 2x2 (scale folded into K3m) ----
                w0 = tmp.tile([128, 800], bf16, tag="w0")
                nc.vector.tensor_add(w0[:], t2[:, 0:800], t2[:, 800:1600])
                nc.vector.tensor_add(w0[:, 0:400], w0[:, 0:400], w0[:, 400:800])
                nc.vector.tensor_add(w0[:, 0:200], w0[:, 0:200], w0[:, 200:400])
                nc.vector.tensor_add(w0[:, 0:100], w0[:, 0:100], w0[:, 100:200])
                u2r = w0[:, 0:100].rearrange("p (i t w) -> p t i w", t=2, w=10)  # i=5
                v2t = tmp.tile([128, 50], bf16, tag="v2t")                 # [5, 10]
                v2r = v2t[:].rearrange("p (i w) -> p i w", w=10)
                nc.vector.tensor_add(v2r, u2r[:, 0], u2r[:, 1])
                v3 = v2t[:].rearrange("p (i j t) -> p t i j", t=2, j=5)
                s2 = tmp.tile([128, 32], bf16, tag="s2")                  # [5,5] in 0:25
                s2r = s2[:, 0:25].rearrange("p (i j) -> p i j", j=5)
                nc.vector.tensor_add(s2r, v3[:, 0], v3[:, 1])

                # ---- transpose s2 -> s2T_all[:, blk*128:...] (feat-major) ----
                pt3 = pstp.tile([128, 128], bf16, tag="ptc")
                nc.tensor.transpose(pt3[0:25, :], s2[:, 0:25], ident[:])
                nc.vector.tensor_copy(s2T_all[0:25, blk * P:(blk + 1) * P], pt3[0:25, :])

            def emit_tail():
                """Batched conv3 + fc1 (feature-major), then per-block fc2."""
                nsplit = (bpc + 511) // 512
                # conv3: [25,120].T @ [25, bpc] -> t3f[0:120]
                ps3 = ps2p.tile([128, 1024], f32, tag="ps2")
                for i in range(nsplit):
                    lo, hi = i * 512, min(bpc, (i + 1) * 512)
                    nc.tensor.matmul(ps3[0:120, lo:hi], k3sb[0:25, :],
                                     s2T_all[0:25, lo:hi])
                nc.scalar.activation(t3f[0:120, 0:bpc], ps3[0:120, 0:bpc], TANH)
                # fc1: [121,84].T @ [121, bpc] -> t4f[0:84]
                ps4 = ps2p.tile([128, 1024], f32, tag="ps2")
                for i in range(nsplit):
                    lo, hi = i * 512, min(bpc, (i + 1) * 512)
                    nc.tensor.matmul(ps4[0:84, lo:hi], fc1sb[0:121, :],
                                     t3f[0:121, lo:hi])
                nc.scalar.activation(t4f[0:84, 0:bpc], ps4[0:84, 0:bpc], TANH)
                # fc2 per block, image-major: out[128,10] = t4f-slice.T @ fc2a
                for blk in range(n_blocks):
                    ps5 = pstp.tile([128, 128], f32, tag="ptc")
                    nc.tensor.matmul(ps5[:, 0:10],
                                     t4f[0:85, blk * P:(blk + 1) * P],
                                     fc2sb[0:85, :])
                    nc.vector.tensor_copy(out_sb[:, blk * 10:(blk + 1) * 10],
                                          ps5[:, 0:10])

            for rep in range(n_reps):
                s1_tiles = {}
                for blk in range(n_blocks):
                    s1_tiles[blk] = emit_front(blk)
                    if blk >= 1:
                        emit_mid(blk - 1, s1_tiles.pop(blk - 1))
                emit_mid(n_blocks - 1, s1_tiles.pop(n_blocks - 1))
                emit_tail()

                # ---- one output DMA: SBUF [128, nblk*10] -> DRAM [nblk*128, 10] ----
                od = out_d[:].rearrange("(blk p) f -> p blk f", p=P)
                ob = out_sb[:].rearrange("p (blk f) -> p blk f", f=10)
                nc.sync.dma_start(od, ob)

    nc.compile()
    return nc


def _get_nc(n_blocks=NBLK, n_reps=1):
    key = ("nc", n_blocks, n_reps)
    if key not in _CACHE:
        _CACHE[key] = _build_bass(n_blocks, n_reps)
    return _CACHE[key]


def kernel(n_reps=1, **inputs):
    x = np.asarray(inputs["x"], np.float32)
    wm = _build_weight_mats(inputs["k1"], inputs["k2"], inputs["k3"],
                            inputs["W1"], inputs["b1"], inputs["W2"], inputs["b2"])
    nc = _get_nc(NBLK, n_reps)

    from concourse.bass_utils import run_bass_kernel_spmd

    in_maps = []
    for core in range(NCORES):
        m = {"x": _prep_x(x, core)}
        m.update(wm)
        in_maps.append(m)

    res = run_bass_kernel_spmd(nc, in_maps, core_ids=list(range(NCORES)))
    _CACHE["last_result"] = res
    out = np.concatenate([r["out"] for r in res.results], axis=0)
    return out.astype(np.float32)
